# revision 4
# baseline (speedup 1.0000x reference)
"""GAT encoder (PyG GATConv-style, single head) for Trainium2, 8 NeuronCores.

Two-pass "project-then-expand" strategy. There is no efficient per-edge
random gather on TRN2 (indirect-DMA is descriptor-bound at ~5-40ns/row),
so per-edge features must be laid out by the host. The v1 kernel expanded
raw x (256B/slot, 58MB/core, DMA-bound at ~220us). v2 cuts the stream 3.8x
by projecting first:

  Pass 1 (device): h_ext = x @ [W | W@att_src | W@att_dst]  -> [N, 34]
     (wext stationary in the PE array, x streams through as moving cols;
      PSUM is DMA'd straight to DRAM, no SBUF staging).
  Host (pure indexing, no model math): gather h_ext rows per edge slot
     into a dst-major, feature-major layout he[d, (c, t, k)] per run.
  Pass 2 (device): per-destination softmax + weighted sum, all DVE/ACT:
     e = lrelu(a_s + a_d), num = exp(e), den = sum_k num,
     msg = h * num (feature-major => all operands innermost-packed bf16
     => DVE 2x_1p fast path), tree-fold over k, then a batched finale
     (fast reciprocal, bias, ACT Sigmoid).

Edges are partitioned by destination across 8 cores (12500 dsts each,
dst = partition lane); per-run slot count S is the max degree within the
run's T_RUN*128 dsts after a global degree sort (12.8% padding).
Precision: h/a_s/a_d shipped bf16, softmax logits f32, accumulation
bf16 tree-fold + f32 den (rel err ~5e-3 vs fp32 reference).
"""
import os
import sys

for _p in ('/opt/trn_rl_repo',):
    if _p not in sys.path and os.path.isdir(_p):
        sys.path.insert(0, _p)

import numpy as np
import ml_dtypes

import concourse.mybir as mybir
import concourse.tile as tile
from concourse import bacc
from concourse.bass_utils import run_bass_kernel_spmd

F32 = mybir.dt.float32
BF16 = mybir.dt.bfloat16

NEG_SLOPE = 0.2
N_CORES = 8
T_RUN = 8          # tiles (of 128 dsts) per run; slot count uniform per run
CW = 34            # projected width: 32 h + a_s + a_d
C_OUT = 32
NEG_BIG = -1.0e9   # a_s fill for dummy slots -> exp == 0

LAST_RESULTS = None
_NC_CACHE = {}


def _plan(src, dst, N, n_cores):
    Nc = N // n_cores
    assert Nc * n_cores == N
    cores = []
    for c in range(n_cores):
        sel = (dst >= c * Nc) & (dst < (c + 1) * Nc)
        s_c, d_c = src[sel], dst[sel] - c * Nc
        not_self = (s_c != d_c + c * Nc).astype(np.int8)
        order = np.lexsort((not_self, d_c))
        srcs_sorted = s_c[order].astype(np.int64)
        counts = np.bincount(d_c, minlength=Nc).astype(np.int64)
        offsets = np.zeros(Nc + 1, np.int64)
        np.cumsum(counts, out=offsets[1:])
        perm = np.argsort(-counts, kind='stable')
        cores.append((srcs_sorted, counts, offsets, perm))

    n_tiles = -(-Nc // 128)
    n_tiles = -(-n_tiles // T_RUN) * T_RUN
    runs = n_tiles // T_RUN
    S_run = np.zeros(runs, np.int64)
    for c in range(n_cores):
        counts, perm = cores[c][1], cores[c][3]
        cnt_sorted = np.ones(n_tiles * 128, np.int64)
        cnt_sorted[:Nc] = counts[perm]
        S_run = np.maximum(S_run, cnt_sorted.reshape(runs, T_RUN * 128).max(axis=1))
    S_run = np.maximum(S_run, 1)
    # run order: smallest first (fast pipeline fill), the big ones after
    rperm = np.concatenate([[runs - 1], np.arange(runs - 1)])
    S_run = S_run[rperm]
    dpads = []
    for c in range(n_cores):
        perm = cores[c][3]
        d_pad = np.full(n_tiles * 128, Nc, np.int64)
        d_pad[:Nc] = perm
        d_pad = d_pad.reshape(runs, T_RUN * 128)[rperm].reshape(-1)
        dpads.append(d_pad)
    return Nc, n_tiles, runs, S_run, cores, dpads


def _build_entries(core_plan, d_pad, Nc, runs, S_run, N):
    """Per-run gather indices ent[r] with shape [T_RUN, S_r, 128] into the
    (N+1)-row h table; row N is the dummy (h=0, a_s=NEG_BIG)."""
    srcs_sorted, counts, offsets, perm = core_plan
    DUMMY = N
    srcs_p = np.concatenate([srcs_sorted, [DUMMY]])
    counts_p = np.concatenate([counts, [1]])
    offsets_p = np.concatenate([offsets, [len(srcs_sorted)]])
    ents = []
    for r in range(runs):
        S = int(S_run[r])
        d = d_pad[r * T_RUN * 128:(r + 1) * T_RUN * 128].reshape(T_RUN, 128)
        k = np.arange(S)
        cnt = counts_p[d]
        pos = offsets_p[d][:, None, :] + k[None, :, None]
        valid = k[None, :, None] < cnt[:, None, :]
        ent = np.full((T_RUN, S, 128), len(srcs_p) - 1, np.int64)
        ent[valid] = np.minimum(pos[valid], len(srcs_p) - 1)
        e = np.where(valid, srcs_p[ent], DUMMY)
        ents.append(e)
    return ents


# ---------------------------------------------------------------- pass 1 ---

NPAD1 = 12800  # 25 * 512 node columns per core (12500 real)


def _build_nc1(n_cores):
    nc = bacc.Bacc("TRN2", target_bir_lowering=False, debug=False,
                   num_devices=n_cores)
    xt = nc.dram_tensor("xt", [128, NPAD1], BF16, kind="ExternalInput").ap()
    wext = nc.dram_tensor("wext", [128, CW], BF16, kind="ExternalInput").ap()
    ht = nc.dram_tensor("ht", [CW, NPAD1], BF16, kind="ExternalOutput").ap()

    CHUNK = 2560  # 5 matmuls of 512 moving columns per input tile
    with tile.TileContext(nc) as tc:
        with (
            tc.tile_pool(name="const", bufs=1) as cpool,
            tc.tile_pool(name="xc", bufs=3) as xpool,
            tc.tile_pool(name="ps", bufs=8, space="PSUM") as pspool,
            tc.tile_pool(name="hs", bufs=3) as hspool,
        ):
            wext_sb = cpool.tile([128, CW], BF16)
            nc.sync.dma_start(wext_sb[:], wext[:])
            for b0 in range(0, NPAD1, CHUNK):
                xc = xpool.tile([128, CHUNK], BF16, tag="xc")
                nc.sync.dma_start(xc[:], xt[:, b0:b0 + CHUNK])
                hs = hspool.tile([128, CHUNK], BF16, tag="hs")
                for j in range(0, CHUNK, 512):
                    ps = pspool.tile([128, 512], F32, tag="ps")
                    nc.tensor.matmul(ps[:CW, :], wext_sb[:],
                                     xc[:, j:j + 512], start=True, stop=True)
                    nc.scalar.copy(hs[:CW, j:j + 512], ps[:CW, :])
                nc.sync.dma_start(ht[:, b0:b0 + CHUNK], hs[:CW, :])
    nc.compile()
    return nc


# ---------------------------------------------------------------- pass 2 ---


def _build_nc2(n_cores, runs, S_run, bias_nonzero):
    nc = bacc.Bacc("TRN2", target_bir_lowering=False, debug=False,
                   num_devices=n_cores)
    total_free = int(CW * T_RUN * S_run.sum())
    he = nc.dram_tensor("he", [128, total_free], BF16, kind="ExternalInput").ap()
    bias = nc.dram_tensor("bias", [128, C_OUT], F32, kind="ExternalInput").ap()
    out = nc.dram_tensor("out", [runs, 128, T_RUN * C_OUT], F32,
                         kind="ExternalOutput").ap()

    T = T_RUN
    Smax = int(max(S_run))
    gp_fold = os.environ.get("GAT_GP", "")
    with tile.TileContext(nc) as tc:
        with (
            tc.tile_pool(name="const", bufs=1) as cpool,
            tc.tile_pool(name="he", bufs=3) as hpool,
            tc.tile_pool(name="msg", bufs=2) as mpool,
            tc.tile_pool(name="work", bufs=2) as wpool,
            tc.tile_pool(name="small", bufs=4) as spool,
        ):
            bias_sb = cpool.tile([128, C_OUT], F32)
            nc.sync.dma_start(bias_sb[:], bias[:])
            outp = cpool.tile([128, runs * T * C_OUT], BF16)
            outf = cpool.tile([128, runs * T * C_OUT], F32)
            den_all = cpool.tile([128, runs * T], F32)

            qbounds = sorted({runs // 4, runs // 2, (3 * runs) // 4, runs})
            base = 0
            for r in range(runs):
                S = int(S_run[r])
                TS = T * S
                nfree = CW * TS
                he_t = hpool.tile([128, CW * T * Smax], BF16, tag="he")
                hev = he_t[:, :nfree]
                nc.sync.dma_start(hev, he[:, base:base + nfree])
                base += nfree

                # views into the feature-major block: [c(34), t(T), k(S)]
                h3 = hev[:, :C_OUT * TS].rearrange(
                    "p (c tk) -> p c tk", tk=TS)          # [128, 32, T*S]
                a_s = hev[:, C_OUT * TS:(C_OUT + 1) * TS]  # [128, T*S]
                a_d = hev[:, (C_OUT + 1) * TS:(C_OUT + 2) * TS] \
                    .rearrange("p (t k) -> p t k", k=S)[:, :, 0:1]

                # e = lrelu(a_s + a_d[dst])  (f32, small)
                e_t = wpool.tile([128, T * Smax], F32, tag="e")
                ev = e_t[:, :TS]
                nc.vector.tensor_tensor(
                    out=ev.rearrange("p (t k) -> p t k", k=S),
                    in0=a_s.rearrange("p (t k) -> p t k", k=S),
                    in1=a_d.to_broadcast([128, T, S]),
                    op=mybir.AluOpType.add)
                nc.vector.scalar_tensor_tensor(
                    out=ev, in0=ev, scalar=NEG_SLOPE, in1=ev,
                    op0=mybir.AluOpType.mult, op1=mybir.AluOpType.max)

                # num = exp(e) (bf16), den = sum_k num (f32)
                num_t = wpool.tile([128, T * Smax], BF16, tag="num")
                nv = num_t[:, :TS]
                nc.scalar.activation(nv, ev, mybir.ActivationFunctionType.Exp)
                nc.vector.reduce_sum(
                    out=den_all[:, r * T:(r + 1) * T],
                    in_=nv.rearrange("p (t k) -> p t k", k=S),
                    axis=mybir.AxisListType.X)

                # msg = h * num  (all bf16, innermost packed -> 2x_1p)
                msg_t = mpool.tile([128, C_OUT * T * Smax], BF16, tag="msg")
                mv = msg_t[:, :C_OUT * TS]
                nc.vector.tensor_tensor(
                    out=mv.rearrange("p (c tk) -> p c tk", tk=TS),
                    in0=h3,
                    in1=nv.rearrange("p (o tk) -> p o tk", o=1)
                        .to_broadcast([128, C_OUT, TS]),
                    op=mybir.AluOpType.mult)

                # tree-fold the k dim: [128, (c t), k]
                m3 = mv.rearrange("p (ct k) -> p ct k", k=S)
                eng = nc.gpsimd if (gp_fold and r % int(gp_fold) == 1) \
                    else nc.vector
                Scur = S
                while Scur > 1:
                    half = Scur // 2
                    eng.tensor_tensor(
                        out=m3[:, :, 0:half],
                        in0=m3[:, :, 0:half],
                        in1=m3[:, :, Scur - half:Scur],
                        op=mybir.AluOpType.add)
                    Scur = Scur - half
                nc.vector.tensor_copy(
                    out=outp[:, r * T * C_OUT:(r + 1) * T * C_OUT],
                    in_=m3[:, :, 0])

                # batched finale once per quarter of runs
                if r + 1 in qbounds:
                    q0 = qbounds[qbounds.index(r + 1) - 1] \
                        if qbounds.index(r + 1) else 0
                    nr = r + 1 - q0
                    dsl = slice(q0 * T, (r + 1) * T)
                    osl = slice(q0 * T * C_OUT, (r + 1) * T * C_OUT)
                    nq = nr * T
                    d2 = spool.tile([128, runs * T], F32, tag="d2")
                    nc.vector.tensor_scalar_max(d2[:, :nq], den_all[:, dsl],
                                                1e-35)
                    rec = spool.tile([128, runs * T], F32, tag="rec")
                    nc.vector.reciprocal_approx_fast(rec[:, :nq], d2[:, :nq])
                    # outp block layout: (r, c, t); rec is (r, t)
                    res4 = outp[:, osl].rearrange(
                        "p (r c t) -> p r c t", r=nr, c=C_OUT)
                    rec_b = rec[:, :nq].rearrange(
                        "p (r o t) -> p r o t", r=nr, o=1) \
                        .to_broadcast([128, nr, C_OUT, T])
                    nc.vector.tensor_tensor(out=res4, in0=res4, in1=rec_b,
                                            op=mybir.AluOpType.mult)
                    if bias_nonzero:
                        bias_b = bias_sb[:].rearrange(
                            "p (r c t) -> p r c t", r=1, t=1) \
                            .to_broadcast([128, nr, C_OUT, T])
                        nc.vector.tensor_tensor(out=res4, in0=res4,
                                                in1=bias_b,
                                                op=mybir.AluOpType.add)
                    nc.scalar.activation(outf[:, osl], outp[:, osl],
                                         mybir.ActivationFunctionType.Sigmoid)
                    nc.sync.dma_start(
                        out[q0:r + 1].transpose([1, 0, 2]),
                        outf[:, osl].rearrange("p (r ct) -> p r ct", r=nr))
    nc.compile()
    return nc


# ------------------------------------------------------------------ host ---


class _Res:
    def __init__(self, exec_time_ns, mean_exec_time_ns):
        self.exec_time_ns = exec_time_ns
        self.mean_exec_time_ns = mean_exec_time_ns


def kernel(x, edge_index, W, att_src, att_dst, bias):
    global LAST_RESULTS
    x = np.asarray(x, np.float32)
    edge_index = np.asarray(edge_index)
    W = np.asarray(W, np.float32)
    att_src = np.asarray(att_src, np.float32)
    att_dst = np.asarray(att_dst, np.float32)
    bias_np = np.asarray(bias, np.float32)

    N, C_in = x.shape
    C_out = W.shape[1]
    assert C_in == 128 and C_out == C_OUT, (C_in, C_out)
    n_cores = N_CORES
    Nc = N // n_cores

    loops = np.arange(N, dtype=np.int64)
    src = np.concatenate([edge_index[0].astype(np.int64), loops])
    dst = np.concatenate([edge_index[1].astype(np.int64), loops])

    Nc, n_tiles, runs, S_run, cores, dpads = _plan(src, dst, N, n_cores)

    ws = (W @ att_src).astype(np.float32)
    wd = (W @ att_dst).astype(np.float32)
    wext = np.concatenate([W, ws[:, None], wd[:, None]],
                          axis=1).astype(ml_dtypes.bfloat16)
    xT = np.ascontiguousarray(x.T).astype(ml_dtypes.bfloat16)  # [128, N]

    key = (n_cores, runs, tuple(S_run.tolist()),
           bool(np.any(bias_np)))
    if key not in _NC_CACHE:
        _NC_CACHE.clear()
        _NC_CACHE[key] = (_build_nc1(n_cores),
                          _build_nc2(n_cores, runs, S_run, bool(np.any(bias_np))))
    nc1, nc2 = _NC_CACHE[key]

    trace = bool(os.environ.get("GAT_TRACE"))

    # ---- pass 1: h_ext = x @ wext on device, node-sharded --------------
    in1 = []
    for c in range(n_cores):
        xt_c = np.zeros((128, NPAD1), ml_dtypes.bfloat16)
        lo, hi = c * Nc, min((c + 1) * Nc, N)
        xt_c[:, :hi - lo] = xT[:, lo:hi]
        in1.append({"xt": xt_c, "wext": wext})
    res1 = run_bass_kernel_spmd(nc1, in1, core_ids=list(range(n_cores)),
                                trace=trace)

    # ---- host: assemble h table, gather per-slot (pure indexing) -------
    h_cat = np.concatenate(
        [np.asarray(res1.results[c]["ht"])[:, :Nc] for c in range(n_cores)],
        axis=1)                                   # [34, N] bf16
    h_rows = np.empty((N + 1, CW), dtype=ml_dtypes.bfloat16)
    h_rows[:N] = h_cat.T
    h_rows[N, :C_OUT] = 0
    h_rows[N, C_OUT] = NEG_BIG      # dummy a_s
    h_rows[N, C_OUT + 1] = 0        # dummy a_d

    bias_bcast = np.broadcast_to(bias_np, (128, C_OUT)).astype(np.float32).copy()
    total_free = int(CW * T_RUN * S_run.sum())
    in2, perms = [], []
    for c in range(n_cores):
        ents = _build_entries(cores[c], dpads[c], Nc, runs, S_run, N)
        he_c = np.empty((128, total_free), ml_dtypes.bfloat16)
        off = 0
        for r in range(runs):
            S = int(S_run[r])
            g = h_rows[ents[r]]                   # [T, S, 128, 34]
            blk = g.transpose(2, 3, 0, 1).reshape(128, CW * T_RUN * S)
            he_c[:, off:off + CW * T_RUN * S] = blk
            off += CW * T_RUN * S
        in2.append({"he": he_c, "bias": bias_bcast})
        perms.append(dpads[c])

    res2 = run_bass_kernel_spmd(nc2, in2, core_ids=list(range(n_cores)),
                                trace=trace)

    t1 = res1.exec_time_ns or 0
    t2 = res2.exec_time_ns or 0
    m1 = res1.mean_exec_time_ns or 0
    m2 = res2.mean_exec_time_ns or 0
    LAST_RESULTS = _Res((t1 + t2) or None, (m1 + m2) or None)

    out_full = np.zeros((N, C_out), np.float32)
    for c in range(n_cores):
        o = np.asarray(res2.results[c]["out"])    # [runs, 128, 32*T] (c,t)
        o = o.reshape(runs, 128, C_out, T_RUN).transpose(0, 3, 1, 2) \
            .reshape(n_tiles * 128, C_out)
        d_pad = perms[c]
        real = d_pad < Nc
        out_full[c * Nc + d_pad[real]] = o[real]
    return out_full


# revision 7
# speedup vs baseline: 1.4640x; 1.4640x over previous
"""GAT encoder (PyG GATConv-style, single head) for Trainium2, 8 NeuronCores.

Two-pass "project-then-expand" strategy. There is no efficient per-edge
random gather on TRN2 (indirect-DMA is descriptor-bound at ~5-40ns/row),
so per-edge features must be laid out by the host. v1 expanded raw x
(256B/slot, 58MB/core, DMA-bound ~220us); v3 projects first and ships only
the 35-value projected bundle per slot (~70B):

  Pass 1 (device): h_ext = x @ [W | W@att_src | W@att_dst] -> [N, 34]
     (wext stationary in the PE array, x streams as moving columns).
  Host (pure indexing): gather the per-slot bundles
     [a_s | a_d | h(32) | 1] into dst-major (c, k, t) layout per run.
     The trailing ones-row makes the softmax denominator fall out of the
     same multiply+fold that aggregates h (row 32 of the fold = den).
  Pass 2 (device): per-dst softmax + weighted sum. dst = partition lane;
     e = a_s + a_d (DVE), lrelu via ACT Prelu and exp via ACT Exp (both
     live in the same activation table -> no table switches), msg =
     h * num with num broadcast on the outer axis (DVE bf16 fast path),
     k-fold with every level a contiguous inner run (no strided tails),
     batched normalize with a fast-reciprocal, one Sigmoid at the end.

Edges are partitioned by destination (12500 dsts/core, degree-sorted so
the per-run slot count S is tight; 12.8% padding at T_RUN=8). Precision:
bundles bf16, logits f32, bf16 tree-fold accumulation (rel err ~5e-3).
"""
import os
import sys

for _p in ('/opt/trn_rl_repo',):
    if _p not in sys.path and os.path.isdir(_p):
        sys.path.insert(0, _p)

import numpy as np
import ml_dtypes

import concourse.mybir as mybir
import concourse.tile as tile
from concourse import bacc
from concourse.bass_utils import run_bass_kernel_spmd

F32 = mybir.dt.float32
BF16 = mybir.dt.bfloat16
ACTF = mybir.ActivationFunctionType

NEG_SLOPE = 0.2
N_CORES = 8
T_RUN = 8          # tiles (of 128 dsts) per run; slot count uniform per run
C_OUT = 32
CB = 33            # folded bundle rows: 32 h + ones (den)
CW = 35            # shipped rows: a_s, a_d, h(32), ones
NEG_BIG = -1.0e9   # a_s fill for dummy slots -> exp == 0

LAST_RESULTS = None
_NC_CACHE = {}


def _plan(src, dst, N, n_cores):
    Nc = N // n_cores
    assert Nc * n_cores == N
    cores = []
    for c in range(n_cores):
        sel = (dst >= c * Nc) & (dst < (c + 1) * Nc)
        s_c, d_c = src[sel], dst[sel] - c * Nc
        not_self = (s_c != d_c + c * Nc).astype(np.int8)
        order = np.lexsort((not_self, d_c))
        srcs_sorted = s_c[order].astype(np.int64)
        counts = np.bincount(d_c, minlength=Nc).astype(np.int64)
        offsets = np.zeros(Nc + 1, np.int64)
        np.cumsum(counts, out=offsets[1:])
        perm = np.argsort(-counts, kind='stable')
        cores.append((srcs_sorted, counts, offsets, perm))

    n_tiles = -(-Nc // 128)
    n_tiles = -(-n_tiles // T_RUN) * T_RUN
    runs = n_tiles // T_RUN
    S_run = np.zeros(runs, np.int64)
    for c in range(n_cores):
        counts, perm = cores[c][1], cores[c][3]
        cnt_sorted = np.ones(n_tiles * 128, np.int64)
        cnt_sorted[:Nc] = counts[perm]
        S_run = np.maximum(S_run, cnt_sorted.reshape(runs, T_RUN * 128).max(axis=1))
    S_run = np.maximum(S_run, 1)
    # run order: smallest first (fast pipeline fill), the big ones after
    rperm = np.concatenate([[runs - 1], np.arange(runs - 1)])
    S_run = S_run[rperm]
    dpads = []
    for c in range(n_cores):
        perm = cores[c][3]
        d_pad = np.full(n_tiles * 128, Nc, np.int64)
        d_pad[:Nc] = perm
        d_pad = d_pad.reshape(runs, T_RUN * 128)[rperm].reshape(-1)
        dpads.append(d_pad)
    return Nc, n_tiles, runs, S_run, cores, dpads


def _build_entries(core_plan, d_pad, Nc, runs, S_run, N):
    """Per-run gather indices ent[r] with shape [T_RUN, S_r, 128] into the
    (N+1)-row bundle table; row N is the dummy."""
    srcs_sorted, counts, offsets, perm = core_plan
    DUMMY = N
    srcs_p = np.concatenate([srcs_sorted, [DUMMY]])
    counts_p = np.concatenate([counts, [1]])
    offsets_p = np.concatenate([offsets, [len(srcs_sorted)]])
    ents = []
    for r in range(runs):
        S = int(S_run[r])
        d = d_pad[r * T_RUN * 128:(r + 1) * T_RUN * 128].reshape(T_RUN, 128)
        k = np.arange(S)
        cnt = counts_p[d]
        pos = offsets_p[d][:, None, :] + k[None, :, None]
        valid = k[None, :, None] < cnt[:, None, :]
        ent = np.full((T_RUN, S, 128), len(srcs_p) - 1, np.int64)
        ent[valid] = np.minimum(pos[valid], len(srcs_p) - 1)
        e = np.where(valid, srcs_p[ent], DUMMY)
        ents.append(e)
    return ents


# ---------------------------------------------------------------- pass 1 ---

NPAD1 = 12800  # 25 * 512 node columns per core (12500 real)
CP = 34        # projected width in pass 1: 32 h + a_s + a_d


def _build_nc1(n_cores):
    nc = bacc.Bacc("TRN2", target_bir_lowering=False, debug=False,
                   num_devices=n_cores)
    xt = nc.dram_tensor("xt", [128, NPAD1], BF16, kind="ExternalInput").ap()
    wext = nc.dram_tensor("wext", [128, CP], BF16, kind="ExternalInput").ap()
    ht = nc.dram_tensor("ht", [CP, NPAD1], BF16, kind="ExternalOutput").ap()

    with tile.TileContext(nc) as tc:
        with (
            tc.tile_pool(name="const", bufs=1) as cpool,
            tc.tile_pool(name="ps", bufs=4, space="PSUM") as pspool,
        ):
            wext_sb = cpool.tile([128, CP], BF16)
            nc.sync.dma_start(wext_sb[:], wext[:])
            xc = cpool.tile([128, NPAD1], BF16)
            for i in range(4):
                sl = slice(i * 3200, (i + 1) * 3200)
                nc.sync.dma_start(xc[:, sl], xt[:, sl])
            hs = cpool.tile([128, NPAD1], BF16)
            for g in range(13):   # 13 groups of (up to) 2 x 512 columns
                b0 = g * 1024
                w = min(1024, NPAD1 - b0)
                ps = pspool.tile([128, 1024], F32, tag="ps")
                for j in range(0, w, 512):
                    nc.tensor.matmul(ps[:CP, j:j + 512], wext_sb[:],
                                     xc[:, b0 + j:b0 + j + 512],
                                     start=True, stop=True)
                eng = nc.scalar if g % 2 == 0 else nc.vector
                if g % 2 == 0:
                    nc.scalar.copy(hs[:CP, b0:b0 + w], ps[:CP, :w])
                else:
                    nc.vector.tensor_copy(out=hs[:CP, b0:b0 + w],
                                          in_=ps[:CP, :w])
                nc.sync.dma_start(ht[:, b0:b0 + w], hs[:CP, b0:b0 + w])
    nc.compile()
    return nc


# ---------------------------------------------------------------- pass 2 ---


def _build_nc2(n_cores, runs, S_run, bias_nonzero):
    nc = bacc.Bacc("TRN2", target_bir_lowering=False, debug=False,
                   num_devices=n_cores)
    total_free = int(CW * T_RUN * S_run.sum())
    he = nc.dram_tensor("he", [128, total_free], BF16, kind="ExternalInput").ap()
    bias = nc.dram_tensor("bias", [128, C_OUT], F32, kind="ExternalInput").ap()
    out = nc.dram_tensor("out", [runs, 128, T_RUN * C_OUT], F32,
                         kind="ExternalOutput").ap()

    T = T_RUN
    Smax = int(max(S_run))
    with tile.TileContext(nc) as tc:
        with (
            tc.tile_pool(name="const", bufs=1) as cpool,
            tc.tile_pool(name="ha", bufs=3) as hapool,
            tc.tile_pool(name="hh", bufs=3) as hhpool,
            tc.tile_pool(name="msg", bufs=3) as mpool,
            tc.tile_pool(name="work", bufs=3) as wpool,
            tc.tile_pool(name="small", bufs=4) as spool,
        ):
            bias_sb = cpool.tile([128, C_OUT], F32)
            nc.sync.dma_start(bias_sb[:], bias[:])
            outp = cpool.tile([128, runs * CB * T], BF16)
            outf = cpool.tile([128, runs * C_OUT * T], F32)

            qbounds = sorted({runs // 4, runs // 2, (3 * runs) // 4, runs})
            base = 0
            for r in range(runs):
                S = int(S_run[r])
                ST = S * T
                # a-part: [a_s | a_d] rows, then h+ones rows, separate DMAs
                ha_t = hapool.tile([128, 2 * T * Smax], BF16, tag="ha")
                hav = ha_t[:, :2 * ST]
                nc.sync.dma_start(hav, he[:, base:base + 2 * ST])
                hh_t = hhpool.tile([128, CB * T * Smax], BF16, tag="hh")
                hhv = hh_t[:, :CB * ST]
                nc.sync.dma_start(hhv, he[:, base + 2 * ST:base + CW * ST])
                base += CW * ST

                # e = a_s + a_d[dst]  (a_d sits at k=0: first T elems)
                e_t = wpool.tile([128, T * Smax], F32, tag="e")
                ev = e_t[:, :ST]
                a_d = hav[:, ST:ST + T].rearrange("p (o t) -> p o t", o=1)
                nc.vector.tensor_tensor(
                    out=ev.rearrange("p (k t) -> p k t", t=T),
                    in0=hav[:, :ST].rearrange("p (k t) -> p k t", t=T),
                    in1=a_d.to_broadcast([128, S, T]),
                    op=mybir.AluOpType.add)
                # lrelu (Prelu) then exp, both ACT, same table
                nc.scalar.activation(ev, ev, ACTF.Prelu, alpha=NEG_SLOPE)
                num_t = wpool.tile([128, T * Smax], BF16, tag="num")
                nv = num_t[:, :ST]
                nc.scalar.activation(nv, ev, ACTF.Exp)

                # msg = [h | 1] * num  (bf16, inner packed, bcast outer)
                msg_t = mpool.tile([128, CB * T * Smax], BF16, tag="msg")
                mv = msg_t[:, :CB * ST]
                nc.vector.tensor_tensor(
                    out=mv.rearrange("p (c kt) -> p c kt", kt=ST),
                    in0=hhv.rearrange("p (c kt) -> p c kt", kt=ST),
                    in1=nv.rearrange("p (o kt) -> p o kt", o=1)
                        .to_broadcast([128, CB, ST]),
                    op=mybir.AluOpType.mult)

                # fold k: every level adds one contiguous [half*T] run
                m3 = mv.rearrange("p (c kt) -> p c kt", kt=ST)
                Scur = S
                while Scur > 1:
                    half = Scur // 2
                    nc.vector.tensor_tensor(
                        out=m3[:, :, 0:half * T],
                        in0=m3[:, :, 0:half * T],
                        in1=m3[:, :, (Scur - half) * T:Scur * T],
                        op=mybir.AluOpType.add)
                    Scur = Scur - half
                # extract folded [c(33), t] block to outp (ACT copy)
                nc.scalar.copy(
                    outp[:, r * CB * T:(r + 1) * CB * T]
                    .rearrange("p (c t) -> p c t", t=T),
                    m3[:, :, 0:T])

                # batched normalize once per quarter of runs
                if r + 1 in qbounds:
                    q0 = qbounds[qbounds.index(r + 1) - 1] \
                        if qbounds.index(r + 1) else 0
                    nr = r + 1 - q0
                    nq = nr * T
                    osl = slice(q0 * CB * T, (r + 1) * CB * T)
                    den_b = outp[:, osl].rearrange(
                        "p (r c t) -> p r c t", r=nr, c=CB)[:, :, C_OUT, :]
                    denf = spool.tile([128, runs * T], F32, tag="denf")
                    nc.scalar.copy(denf[:, :nq]
                                   .rearrange("p (r t) -> p r t", t=T), den_b)
                    nc.vector.tensor_scalar_max(denf[:, :nq], denf[:, :nq],
                                                1e-35)
                    rec = spool.tile([128, runs * T], F32, tag="rec")
                    nc.vector.reciprocal_approx_fast(rec[:, :nq], denf[:, :nq])
                    res4 = outp[:, osl].rearrange(
                        "p (r c t) -> p r c t", r=nr, c=CB)[:, :, 0:C_OUT, :]
                    rec_b = rec[:, :nq].rearrange(
                        "p (r o t) -> p r o t", r=nr, o=1) \
                        .to_broadcast([128, nr, C_OUT, T])
                    nc.vector.tensor_tensor(out=res4, in0=res4, in1=rec_b,
                                            op=mybir.AluOpType.mult)
                    if bias_nonzero:
                        bias_b = bias_sb[:].rearrange(
                            "p (r c t) -> p r c t", r=1, t=1) \
                            .to_broadcast([128, nr, C_OUT, T])
                        nc.vector.tensor_tensor(out=res4, in0=res4,
                                                in1=bias_b,
                                                op=mybir.AluOpType.add)

            # single sigmoid + output DMA at the end (one table switch)
            allres = outp[:].rearrange(
                "p (r c t) -> p r c t", c=CB, t=T)[:, :, 0:C_OUT, :]
            nc.scalar.activation(
                outf[:].rearrange("p (r c t) -> p r c t", c=C_OUT, t=T),
                allres, ACTF.Sigmoid)
            nc.sync.dma_start(
                out[:].transpose([1, 0, 2]),
                outf[:].rearrange("p (r ct) -> p r ct", r=runs))
    nc.compile()
    return nc


# ------------------------------------------------------------------ host ---


class _Res:
    def __init__(self, exec_time_ns, mean_exec_time_ns):
        self.exec_time_ns = exec_time_ns
        self.mean_exec_time_ns = mean_exec_time_ns


def kernel(x, edge_index, W, att_src, att_dst, bias):
    global LAST_RESULTS
    x = np.asarray(x, np.float32)
    edge_index = np.asarray(edge_index)
    W = np.asarray(W, np.float32)
    att_src = np.asarray(att_src, np.float32)
    att_dst = np.asarray(att_dst, np.float32)
    bias_np = np.asarray(bias, np.float32)

    N, C_in = x.shape
    C_out = W.shape[1]
    assert C_in == 128 and C_out == C_OUT, (C_in, C_out)
    n_cores = N_CORES
    Nc = N // n_cores

    loops = np.arange(N, dtype=np.int64)
    src = np.concatenate([edge_index[0].astype(np.int64), loops])
    dst = np.concatenate([edge_index[1].astype(np.int64), loops])

    Nc, n_tiles, runs, S_run, cores, dpads = _plan(src, dst, N, n_cores)

    ws = (W @ att_src).astype(np.float32)
    wd = (W @ att_dst).astype(np.float32)
    wext = np.concatenate([W, ws[:, None], wd[:, None]],
                          axis=1).astype(ml_dtypes.bfloat16)
    xT = np.ascontiguousarray(x.T).astype(ml_dtypes.bfloat16)  # [128, N]

    key = (n_cores, runs, tuple(S_run.tolist()), bool(np.any(bias_np)))
    if key not in _NC_CACHE:
        _NC_CACHE.clear()
        _NC_CACHE[key] = (_build_nc1(n_cores),
                          _build_nc2(n_cores, runs, S_run,
                                     bool(np.any(bias_np))))
    nc1, nc2 = _NC_CACHE[key]

    trace = bool(os.environ.get("GAT_TRACE"))

    # ---- pass 1: h_ext = x @ wext on device, node-sharded --------------
    in1 = []
    for c in range(n_cores):
        xt_c = np.zeros((128, NPAD1), ml_dtypes.bfloat16)
        lo, hi = c * Nc, min((c + 1) * Nc, N)
        xt_c[:, :hi - lo] = xT[:, lo:hi]
        in1.append({"xt": xt_c, "wext": wext})
    res1 = run_bass_kernel_spmd(nc1, in1, core_ids=list(range(n_cores)),
                                trace=trace)

    # ---- host: assemble bundle table, gather (pure indexing) -----------
    h_cat = np.concatenate(
        [np.asarray(res1.results[c]["ht"])[:, :Nc] for c in range(n_cores)],
        axis=1)                                   # [34, N] bf16
    # bundle rows: [a_s | a_d | h(32) | ones]
    h_rows = np.empty((N + 1, CW), dtype=ml_dtypes.bfloat16)
    h_rows[:N, 0] = h_cat[32]
    h_rows[:N, 1] = h_cat[33]
    h_rows[:N, 2:2 + C_OUT] = h_cat[:32].T
    h_rows[:N, 34] = 1.0
    h_rows[N] = 0
    h_rows[N, 0] = NEG_BIG       # dummy a_s
    h_rows[N, 34] = 1.0

    bias_bcast = np.broadcast_to(bias_np, (128, C_OUT)).astype(np.float32).copy()
    total_free = int(CW * T_RUN * S_run.sum())
    in2, perms = [], []
    for c in range(n_cores):
        ents = _build_entries(cores[c], dpads[c], Nc, runs, S_run, N)
        he_c = np.empty((128, total_free), ml_dtypes.bfloat16)
        off = 0
        for r in range(runs):
            S = int(S_run[r])
            g = h_rows[ents[r]]                   # [T, S, 128, 35]
            blk = g.transpose(2, 3, 1, 0).reshape(128, CW * S * T_RUN)
            he_c[:, off:off + CW * S * T_RUN] = blk
            off += CW * S * T_RUN
        in2.append({"he": he_c, "bias": bias_bcast})
        perms.append(dpads[c])

    res2 = run_bass_kernel_spmd(nc2, in2, core_ids=list(range(n_cores)),
                                trace=trace)

    t1 = res1.exec_time_ns or 0
    t2 = res2.exec_time_ns or 0
    m1 = res1.mean_exec_time_ns or 0
    m2 = res2.mean_exec_time_ns or 0
    LAST_RESULTS = _Res((t1 + t2) or None, (m1 + m2) or None)

    out_full = np.zeros((N, C_out), np.float32)
    for c in range(n_cores):
        o = np.asarray(res2.results[c]["out"])    # [runs, 128, 32*T] (c,t)
        o = o.reshape(runs, 128, C_out, T_RUN).transpose(0, 3, 1, 2) \
            .reshape(n_tiles * 128, C_out)
        d_pad = perms[c]
        real = d_pad < Nc
        out_full[c * Nc + d_pad[real]] = o[real]
    return out_full


# revision 11
# speedup vs baseline: 1.5308x; 1.0456x over previous
"""GAT encoder (PyG GATConv-style, single head) for Trainium2, 8 NeuronCores.

Two-pass "project-then-expand" strategy. There is no efficient per-edge
random gather on TRN2 (indirect-DMA is descriptor-bound at ~5-40ns/row),
so per-edge features must be laid out by the host. v1 expanded raw x
(256B/slot, 58MB/core, DMA-bound ~220us); v3 projects first and ships only
the 35-value projected bundle per slot (~70B):

  Pass 1 (device): h_ext = x @ [W | W@att_src | W@att_dst] -> [N, 34]
     (wext stationary in the PE array, x streams as moving columns).
  Host (pure indexing): gather the per-slot bundles
     [a_s | a_d | h(32) | 1] into dst-major (c, k, t) layout per run.
     The trailing ones-row makes the softmax denominator fall out of the
     same multiply+fold that aggregates h (row 32 of the fold = den).
  Pass 2 (device): per-dst softmax + weighted sum. dst = partition lane;
     e = a_s + a_d (DVE), lrelu via ACT Prelu and exp via ACT Exp (both
     live in the same activation table -> no table switches), msg =
     h * num with num broadcast on the outer axis (DVE bf16 fast path),
     k-fold with every level a contiguous inner run (no strided tails),
     batched normalize with a fast-reciprocal, one Sigmoid at the end.

Edges are partitioned by destination (12500 dsts/core, degree-sorted so
the per-run slot count S is tight; 12.8% padding at T_RUN=8). Precision:
bundles bf16, logits f32, bf16 tree-fold accumulation (rel err ~5e-3).
"""
import os
import sys

for _p in ('/opt/trn_rl_repo',):
    if _p not in sys.path and os.path.isdir(_p):
        sys.path.insert(0, _p)

import numpy as np
import ml_dtypes

import concourse.mybir as mybir
import concourse.tile as tile
from concourse import bacc
from concourse.bass_utils import run_bass_kernel_spmd

F32 = mybir.dt.float32
BF16 = mybir.dt.bfloat16
ACTF = mybir.ActivationFunctionType

NEG_SLOPE = 0.2
N_CORES = 8
T_RUN = 8          # tiles (of 128 dsts) per run; slot count uniform per run
C_OUT = 32
CB = 33            # folded bundle rows: 32 h + ones (den)
CW = 35            # shipped rows: a_s, a_d, h(32), ones
NEG_BIG = -1.0e9   # a_s fill for dummy slots -> exp == 0

LAST_RESULTS = None
_NC_CACHE = {}


def _plan(src, dst, N, n_cores):
    Nc = N // n_cores
    assert Nc * n_cores == N
    cores = []
    for c in range(n_cores):
        sel = (dst >= c * Nc) & (dst < (c + 1) * Nc)
        s_c, d_c = src[sel], dst[sel] - c * Nc
        not_self = (s_c != d_c + c * Nc).astype(np.int8)
        order = np.lexsort((not_self, d_c))
        srcs_sorted = s_c[order].astype(np.int64)
        counts = np.bincount(d_c, minlength=Nc).astype(np.int64)
        offsets = np.zeros(Nc + 1, np.int64)
        np.cumsum(counts, out=offsets[1:])
        perm = np.argsort(-counts, kind='stable')
        cores.append((srcs_sorted, counts, offsets, perm))

    n_tiles = -(-Nc // 128)
    n_tiles = -(-n_tiles // T_RUN) * T_RUN
    runs = n_tiles // T_RUN
    S_run = np.zeros(runs, np.int64)
    for c in range(n_cores):
        counts, perm = cores[c][1], cores[c][3]
        cnt_sorted = np.ones(n_tiles * 128, np.int64)
        cnt_sorted[:Nc] = counts[perm]
        S_run = np.maximum(S_run, cnt_sorted.reshape(runs, T_RUN * 128).max(axis=1))
    S_run = np.maximum(S_run, 1)
    # run order: smallest first (fast pipeline fill), the big ones after
    rperm = np.concatenate([[runs - 1], np.arange(runs - 1)])
    S_run = S_run[rperm]
    dpads = []
    for c in range(n_cores):
        perm = cores[c][3]
        d_pad = np.full(n_tiles * 128, Nc, np.int64)
        d_pad[:Nc] = perm
        d_pad = d_pad.reshape(runs, T_RUN * 128)[rperm].reshape(-1)
        dpads.append(d_pad)
    return Nc, n_tiles, runs, S_run, cores, dpads


def _build_entries(core_plan, d_pad, Nc, runs, S_run, N):
    """Per-run gather indices ent[r] with shape [T_RUN, S_r, 128] into the
    (N+1)-row bundle table; row N is the dummy."""
    srcs_sorted, counts, offsets, perm = core_plan
    DUMMY = N
    srcs_p = np.concatenate([srcs_sorted, [DUMMY]])
    counts_p = np.concatenate([counts, [1]])
    offsets_p = np.concatenate([offsets, [len(srcs_sorted)]])
    ents = []
    for r in range(runs):
        S = int(S_run[r])
        d = d_pad[r * T_RUN * 128:(r + 1) * T_RUN * 128].reshape(T_RUN, 128)
        k = np.arange(S)
        cnt = counts_p[d]
        pos = offsets_p[d][:, None, :] + k[None, :, None]
        valid = k[None, :, None] < cnt[:, None, :]
        ent = np.full((T_RUN, S, 128), len(srcs_p) - 1, np.int64)
        ent[valid] = np.minimum(pos[valid], len(srcs_p) - 1)
        e = np.where(valid, srcs_p[ent], DUMMY)
        ents.append(e)
    return ents


# ---------------------------------------------------------------- pass 1 ---

NPAD1 = 12800  # 25 * 512 node columns per core (12500 real)
CP = 34        # projected width in pass 1: 32 h + a_s + a_d


def _build_nc1(n_cores):
    nc = bacc.Bacc("TRN2", target_bir_lowering=False, debug=False,
                   num_devices=n_cores)
    xt = nc.dram_tensor("xt", [128, NPAD1], BF16, kind="ExternalInput").ap()
    wext = nc.dram_tensor("wext", [128, CP], BF16, kind="ExternalInput").ap()
    ht = nc.dram_tensor("ht", [CP, NPAD1], BF16, kind="ExternalOutput").ap()

    CHUNK = 2048
    with tile.TileContext(nc) as tc:
        with (
            tc.tile_pool(name="const", bufs=1) as cpool,
            tc.tile_pool(name="xc", bufs=3) as xpool,
            tc.tile_pool(name="ps", bufs=2, space="PSUM") as pspool,
        ):
            wext_sb = cpool.tile([128, CP], BF16)
            nc.sync.dma_start(wext_sb[:], wext[:])
            hs = cpool.tile([128, NPAD1], BF16)
            for g, b0 in enumerate(range(0, NPAD1, CHUNK)):
                w = min(CHUNK, NPAD1 - b0)
                xc = xpool.tile([128, CHUNK], BF16, tag="xc")
                nc.sync.dma_start(xc[:, :w], xt[:, b0:b0 + w])
                ps = pspool.tile([128, CHUNK], F32, tag="ps")
                for j in range(0, w, 512):
                    nc.tensor.matmul(ps[:CP, j:j + 512], wext_sb[:],
                                     xc[:, j:j + 512], start=True, stop=True)
                if g % 2 == 0:
                    nc.scalar.copy(hs[:CP, b0:b0 + w], ps[:CP, :w])
                else:
                    nc.vector.tensor_copy(out=hs[:CP, b0:b0 + w],
                                          in_=ps[:CP, :w])
            half = (NPAD1 // CHUNK // 2) * CHUNK
            nc.sync.dma_start(ht[:, :half], hs[:CP, :half])
            nc.sync.dma_start(ht[:, half:], hs[:CP, half:])
    nc.compile()
    return nc


# ---------------------------------------------------------------- pass 2 ---


def _build_nc2(n_cores, runs, S_run, bias_nonzero):
    nc = bacc.Bacc("TRN2", target_bir_lowering=False, debug=False,
                   num_devices=n_cores)
    total_free = int(CW * T_RUN * S_run.sum())
    he = nc.dram_tensor("he", [128, total_free], BF16, kind="ExternalInput").ap()
    bias = nc.dram_tensor("bias", [128, C_OUT], F32, kind="ExternalInput").ap()
    out = nc.dram_tensor("out", [runs, 128, T_RUN * C_OUT], F32,
                         kind="ExternalOutput").ap()

    T = T_RUN
    Smax = int(max(S_run))
    with tile.TileContext(nc) as tc:
        with (
            tc.tile_pool(name="const", bufs=1) as cpool,
            tc.tile_pool(name="ha", bufs=6) as hapool,
            tc.tile_pool(name="hh", bufs=3) as hhpool,
            tc.tile_pool(name="msg", bufs=3) as mpool,
            tc.tile_pool(name="work", bufs=4) as wpool,
            tc.tile_pool(name="small", bufs=4) as spool,
        ):
            bias_sb = cpool.tile([128, C_OUT], F32)
            nc.sync.dma_start(bias_sb[:], bias[:])
            outp = cpool.tile([128, runs * CB * T], BF16)
            outf = cpool.tile([128, runs * C_OUT * T], F32)

            qbounds = sorted({runs // 2, (3 * runs) // 4, runs})
            base = 0
            for r in range(runs):
                S = int(S_run[r])
                ST = S * T
                # a-part: [a_s | a_d] rows, then h+ones rows, separate DMAs
                ha_t = hapool.tile([128, 2 * T * Smax], BF16, tag="ha")
                hav = ha_t[:, :2 * ST]
                nc.sync.dma_start(hav, he[:, base:base + 2 * ST])
                hh_t = hhpool.tile([128, CB * T * Smax], BF16, tag="hh")
                hhv = hh_t[:, :CB * ST]
                nc.sync.dma_start(hhv, he[:, base + 2 * ST:base + CW * ST])
                base += CW * ST

                # e = a_s + a_d[dst]  (a_d sits at k=0: first T elems)
                e_t = wpool.tile([128, T * Smax], F32, tag="e")
                ev = e_t[:, :ST]
                a_d = hav[:, ST:ST + T].rearrange("p (o t) -> p o t", o=1)
                nc.vector.tensor_tensor(
                    out=ev.rearrange("p (k t) -> p k t", t=T),
                    in0=hav[:, :ST].rearrange("p (k t) -> p k t", t=T),
                    in1=a_d.to_broadcast([128, S, T]),
                    op=mybir.AluOpType.add)
                # lrelu (Prelu) then exp, both ACT, same table
                nc.scalar.activation(ev, ev, ACTF.Prelu, alpha=NEG_SLOPE)
                num_t = wpool.tile([128, T * Smax], BF16, tag="num")
                nv = num_t[:, :ST]
                nc.scalar.activation(nv, ev, ACTF.Exp)

                # msg = [h | 1] * num  (bf16, inner packed, bcast outer)
                msg_t = mpool.tile([128, CB * T * Smax], BF16, tag="msg")
                mv = msg_t[:, :CB * ST]
                nc.vector.tensor_tensor(
                    out=mv.rearrange("p (c kt) -> p c kt", kt=ST),
                    in0=hhv.rearrange("p (c kt) -> p c kt", kt=ST),
                    in1=nv.rearrange("p (o kt) -> p o kt", o=1)
                        .to_broadcast([128, CB, ST]),
                    op=mybir.AluOpType.mult)

                # fold k: every level adds one contiguous [half*T] run;
                # the last level writes straight into outp
                m3 = mv.rearrange("p (c kt) -> p c kt", kt=ST)
                out_blk = outp[:, r * CB * T:(r + 1) * CB * T] \
                    .rearrange("p (c t) -> p c t", t=T)
                Scur = S
                while Scur > 2:
                    half = Scur // 2
                    nc.vector.tensor_tensor(
                        out=m3[:, :, 0:half * T],
                        in0=m3[:, :, 0:half * T],
                        in1=m3[:, :, (Scur - half) * T:Scur * T],
                        op=mybir.AluOpType.add)
                    Scur = Scur - half
                if Scur == 2:
                    nc.vector.tensor_tensor(
                        out=out_blk, in0=m3[:, :, 0:T], in1=m3[:, :, T:2 * T],
                        op=mybir.AluOpType.add)
                else:
                    nc.vector.tensor_copy(out=out_blk, in_=m3[:, :, 0:T])

                # staged finalize: normalize, sigmoid, and ship a block
                if r + 1 in qbounds:
                    q0 = qbounds[qbounds.index(r + 1) - 1] \
                        if qbounds.index(r + 1) else 0
                    nr = r + 1 - q0
                    nq = nr * T
                    osl = slice(q0 * CB * T, (r + 1) * CB * T)
                    den_b = outp[:, osl].rearrange(
                        "p (r c t) -> p r c t", r=nr, c=CB)[:, :, C_OUT, :]
                    denf = spool.tile([128, runs * T], F32, tag="denf")
                    nc.vector.tensor_copy(
                        out=denf[:, :nq].rearrange("p (r t) -> p r t", t=T),
                        in_=den_b)
                    nc.vector.tensor_scalar_max(denf[:, :nq], denf[:, :nq],
                                                1e-35)
                    rec = spool.tile([128, runs * T], F32, tag="rec")
                    nc.vector.reciprocal_approx_fast(rec[:, :nq], denf[:, :nq])
                    res4 = outp[:, osl].rearrange(
                        "p (r c t) -> p r c t", r=nr, c=CB)[:, :, 0:C_OUT, :]
                    rec_b = rec[:, :nq].rearrange(
                        "p (r o t) -> p r o t", r=nr, o=1) \
                        .to_broadcast([128, nr, C_OUT, T])
                    nc.vector.tensor_tensor(out=res4, in0=res4, in1=rec_b,
                                            op=mybir.AluOpType.mult)
                    if bias_nonzero:
                        bias_b = bias_sb[:].rearrange(
                            "p (r c t) -> p r c t", r=1, t=1) \
                            .to_broadcast([128, nr, C_OUT, T])
                        nc.vector.tensor_tensor(out=res4, in0=res4,
                                                in1=bias_b,
                                                op=mybir.AluOpType.add)
                    fsl = slice(q0 * C_OUT * T, (r + 1) * C_OUT * T)
                    nc.scalar.activation(
                        outf[:, fsl].rearrange(
                            "p (r c t) -> p r c t", c=C_OUT, t=T),
                        res4, ACTF.Sigmoid)
                    nc.sync.dma_start(
                        out[q0:r + 1].transpose([1, 0, 2]),
                        outf[:, fsl].rearrange("p (r ct) -> p r ct", r=nr))
    nc.compile()
    return nc


# ------------------------------------------------------------------ host ---


class _Res:
    def __init__(self, exec_time_ns, mean_exec_time_ns):
        self.exec_time_ns = exec_time_ns
        self.mean_exec_time_ns = mean_exec_time_ns


def kernel(x, edge_index, W, att_src, att_dst, bias):
    global LAST_RESULTS
    x = np.asarray(x, np.float32)
    edge_index = np.asarray(edge_index)
    W = np.asarray(W, np.float32)
    att_src = np.asarray(att_src, np.float32)
    att_dst = np.asarray(att_dst, np.float32)
    bias_np = np.asarray(bias, np.float32)

    N, C_in = x.shape
    C_out = W.shape[1]
    assert C_in == 128 and C_out == C_OUT, (C_in, C_out)
    n_cores = N_CORES
    Nc = N // n_cores

    loops = np.arange(N, dtype=np.int64)
    src = np.concatenate([edge_index[0].astype(np.int64), loops])
    dst = np.concatenate([edge_index[1].astype(np.int64), loops])

    Nc, n_tiles, runs, S_run, cores, dpads = _plan(src, dst, N, n_cores)

    ws = (W @ att_src).astype(np.float32)
    wd = (W @ att_dst).astype(np.float32)
    wext = np.concatenate([W, ws[:, None], wd[:, None]],
                          axis=1).astype(ml_dtypes.bfloat16)
    xT = np.ascontiguousarray(x.T).astype(ml_dtypes.bfloat16)  # [128, N]

    key = (n_cores, runs, tuple(S_run.tolist()), bool(np.any(bias_np)))
    if key not in _NC_CACHE:
        _NC_CACHE.clear()
        _NC_CACHE[key] = (_build_nc1(n_cores),
                          _build_nc2(n_cores, runs, S_run,
                                     bool(np.any(bias_np))))
    nc1, nc2 = _NC_CACHE[key]

    trace = bool(os.environ.get("GAT_TRACE"))

    # ---- pass 1: h_ext = x @ wext on device, node-sharded --------------
    in1 = []
    for c in range(n_cores):
        xt_c = np.zeros((128, NPAD1), ml_dtypes.bfloat16)
        lo, hi = c * Nc, min((c + 1) * Nc, N)
        xt_c[:, :hi - lo] = xT[:, lo:hi]
        in1.append({"xt": xt_c, "wext": wext})
    res1 = run_bass_kernel_spmd(nc1, in1, core_ids=list(range(n_cores)),
                                trace=trace)

    # ---- host: assemble bundle table, gather (pure indexing) -----------
    h_cat = np.concatenate(
        [np.asarray(res1.results[c]["ht"])[:, :Nc] for c in range(n_cores)],
        axis=1)                                   # [34, N] bf16
    # bundle rows: [a_s | a_d | h(32) | ones]
    h_rows = np.empty((N + 1, CW), dtype=ml_dtypes.bfloat16)
    h_rows[:N, 0] = h_cat[32]
    h_rows[:N, 1] = h_cat[33]
    h_rows[:N, 2:2 + C_OUT] = h_cat[:32].T
    h_rows[:N, 34] = 1.0
    h_rows[N] = 0
    h_rows[N, 0] = NEG_BIG       # dummy a_s
    h_rows[N, 34] = 1.0

    bias_bcast = np.broadcast_to(bias_np, (128, C_OUT)).astype(np.float32).copy()
    total_free = int(CW * T_RUN * S_run.sum())
    in2, perms = [], []
    for c in range(n_cores):
        ents = _build_entries(cores[c], dpads[c], Nc, runs, S_run, N)
        he_c = np.empty((128, total_free), ml_dtypes.bfloat16)
        off = 0
        for r in range(runs):
            S = int(S_run[r])
            g = h_rows[ents[r]]                   # [T, S, 128, 35]
            blk = g.transpose(2, 3, 1, 0).reshape(128, CW * S * T_RUN)
            he_c[:, off:off + CW * S * T_RUN] = blk
            off += CW * S * T_RUN
        in2.append({"he": he_c, "bias": bias_bcast})
        perms.append(dpads[c])

    res2 = run_bass_kernel_spmd(nc2, in2, core_ids=list(range(n_cores)),
                                trace=trace)

    t1 = res1.exec_time_ns or 0
    t2 = res2.exec_time_ns or 0
    m1 = res1.mean_exec_time_ns or 0
    m2 = res2.mean_exec_time_ns or 0
    LAST_RESULTS = _Res((t1 + t2) or None, (m1 + m2) or None)

    out_full = np.zeros((N, C_out), np.float32)
    for c in range(n_cores):
        o = np.asarray(res2.results[c]["out"])    # [runs, 128, 32*T] (c,t)
        o = o.reshape(runs, 128, C_out, T_RUN).transpose(0, 3, 1, 2) \
            .reshape(n_tiles * 128, C_out)
        d_pad = perms[c]
        real = d_pad < Nc
        out_full[c * Nc + d_pad[real]] = o[real]
    return out_full
